# revision 30
# baseline (speedup 1.0000x reference)
"""Single-head causal attention (B=8, T=2048, C=1024, H=128) on 8 TRN2 NeuronCores.

Sharding: data-parallel over batch — core b computes batch element b entirely
(no collectives). Host pre-transposes x[b] to xT=[C,T] in float16 (10 mantissa
bits keep logit error ~7e-3; bf16 would not); the device returns out^T=[H,T]
which the host transposes back and adds bv to.

Per-core dataflow (T split into 4 chunks of 512 columns):
  qT/kT chunk   = sum_c W[c-tile].T @ xT[c-tile]    (fp16 matmuls, f32 PSUM)
  bq added on DVE; bk dropped (softmax is invariant to per-query shifts);
  bv applied on the host after the gather.
  v_nat[s-tile] = sum_c xT[c,s-slice].T @ Wv[c]     (fp16, natural [s,h] layout
                  directly — fp16 runs 1 cyc/row at any free size, so no
                  PE transpose pass is needed)
  scoresT[s,t]  = kT-tile.T-as-lhsT @ qT-chunk      (float32r, causal-trimmed,
                  diagonal tiles clamped to >=256 free so fp32r stays 1 cyc/row)
  expT          = Exp(scoresT) on ACT -> bf16, diagonal blocks masked on DVE
  denom         = DVE-accumulated bf16 expT tiles + one ones-matmul per chunk
  out^T        += v_nat[i] @ expT                   (bf16 matmuls)
  out^T[:,chunk]= out^T * 1/denom  -> DMA out

Scheduling notes (the perf comes from these):
  - one dma_start per logical tensor (each costs ~0.7us of serial Sync time);
    all four x-chunks are enqueued in the prologue (SBUF-resident).
  - warmup matmuls bridge the DMA prologue so the PE clock (DVFS) is ramped
    when real work starts; any PE idle gap downclocks and costs double.
  - chunk j+1's projections ride a filler queue interleaved into chunk j's
    exp-latency-bound attention ops, drained adaptively; each chunk's
    denominator/normalize epilogue is deferred into the next chunk's stream;
    the next chunk's off-diagonal scores are pre-emitted (PRE) so ACT is never
    the serial pacer at a chunk boundary.
"""

import os
from collections import deque

import numpy as np

T, C, H = 2048, 1024, 128
B = 8
P = 128
CT = C // P          # 8 contraction tiles
NCH = 4              # t-chunks
CHW = T // NCH       # 512 chunk width
SPC = CHW // P       # 4 s-tiles per chunk
MW = 256             # uniform mask width
N_CORES = 8
N_WARM = 26

LAST_EXEC_TIME_NS = None

_BUILT = None


def _build():
    global _BUILT
    if _BUILT is not None:
        return _BUILT

    import concourse.bass as bass  # noqa: F401
    import concourse.mybir as mybir
    from concourse import bacc
    from concourse.tile import TileContext

    F32 = mybir.dt.float32
    F32R = mybir.dt.float32r
    F16 = mybir.dt.float16
    BF16 = mybir.dt.bfloat16
    Exp = mybir.ActivationFunctionType.Exp
    ADD = mybir.AluOpType.add
    MULT = mybir.AluOpType.mult

    nc = bacc.Bacc()

    xT_ext = nc.declare_dram_parameter("xT", [C, T], F16, isOutput=False)
    w_ext = {
        n: nc.declare_dram_parameter(n, [C, H], F16, isOutput=False)
        for n in ("Wq", "Wk", "Wv")
    }
    bq_ext = nc.declare_dram_parameter("bq", [H, 1], F32, isOutput=False)
    masks_ext = nc.declare_dram_parameter("masks", [P, SPC * MW], BF16, isOutput=False)
    out_ext = nc.declare_dram_parameter("out", [H, T], F32, isOutput=True)

    xT_r = xT_ext.rearrange("(ct p) t -> p ct t", p=P)
    w_r = {n: w_ext[n].rearrange("(ct p) h -> p ct h", p=P) for n in w_ext}

    with TileContext(nc) as tc:
        with (
            tc.tile_pool(name="const", bufs=1) as const,
            tc.tile_pool(name="xch", bufs=3) as x_pool,
            tc.tile_pool(name="qch", bufs=2) as q_pool,
            tc.tile_pool(name="ktc", bufs=4) as kt_pool,
            tc.tile_pool(name="vn", bufs=4) as vn_pool,
            tc.tile_pool(name="ex", bufs=10) as e_pool,
            tc.tile_pool(name="ex2", bufs=6) as e2_pool,
            tc.tile_pool(name="accp", bufs=2) as acc_pool,
            tc.tile_pool(name="outp", bufs=2) as out_pool,
            tc.tile_pool(name="ps_proj", bufs=2, space="PSUM") as proj_ps,
            tc.tile_pool(name="ps_sc", bufs=2, space="PSUM") as sc_ps,
            tc.tile_pool(name="ps_o", bufs=2, space="PSUM") as o_ps,
        ):
            # ---- constants / prologue DMAs (priority order) ----
            # batched transfers: one dma_start per logical tensor, because
            # every dma_start costs ~0.7us of serial Sync-engine issue time
            w_t = {}
            w_t["Wq"] = const.tile([P, CT, H], F16, tag="w_Wq", name="w_Wq")
            nc.sync.dma_start(w_t["Wq"][:], w_r["Wq"][:])
            xt0 = const.tile([P, CT, CHW], F16, tag="x0", name="x_0")
            nc.sync.dma_start(xt0[:], xT_r[:, :, 0:CHW])
            bq_sb = const.tile([H, 1], F32, tag="bq", name="bq_sb")
            nc.sync.dma_start(bq_sb[:], bq_ext[:])
            w_t["Wk"] = const.tile([P, CT, H], F16, tag="w_Wk", name="w_Wk")
            nc.sync.dma_start(w_t["Wk"][:], w_r["Wk"][:])
            mi = const.tile([P, SPC * MW], BF16, tag="mi", name="mi")
            nc.sync.dma_start(mi[:], masks_ext[:])
            xt1 = x_pool.tile([P, CT, CHW], F16, tag="x", name="x_1")
            nc.sync.dma_start(xt1[:], xT_r[:, :, CHW : 2 * CHW])
            w_t["Wv"] = const.tile([P, CT, H], F16, tag="w_Wv", name="w_Wv")
            nc.sync.dma_start(w_t["Wv"][:], w_r["Wv"][:])
            ones_r = const.tile([P, P], BF16, tag="ones_r", name="ones_r")
            nc.vector.memset(ones_r[:], 1.0)
            x23_tiles = {}
            for jj in (2, 3):
                xt = x_pool.tile([P, CT, CHW], F16, tag="x", name=f"x_{jj}")
                nc.sync.dma_start(xt[:], xT_r[:, :, CHW * jj : CHW * (jj + 1)])
                x23_tiles[jj] = xt
            warm_src = const.tile([P, CHW], BF16, tag="warm_src", name="warm_src")
            nc.vector.memset(warm_src[:], 0.0)

            # PE warmup: dummy matmuls spanning the DMA prologue so HAM is at
            # full clock when the first real matmul issues.
            ps_warm = proj_ps.tile([P, CHW], F32, tag="proj", name="ps_warm")
            for _w in range(N_WARM):
                nc.tensor.matmul(
                    ps_warm[:], warm_src[:, 0:P], warm_src[:], start=True, stop=True,
                )

            kt_ch = [None] * NCH   # [h=128, s=512] f32r per chunk
            v_nat = [None] * NCH   # [s=128, st*128+h] bf16 per chunk
            q_ch = [None] * NCH    # [h=128, t=512] f32r per chunk

            filler = deque()

            def pop_fillers(k):
                for _ in range(min(k, len(filler))):
                    filler.popleft()()

            def drain_fillers():
                while filler:
                    filler.popleft()()

            def proj_q_thunks(j, xf):
                st8 = {}

                def mk_mm(c):
                    def run():
                        if c == 0:
                            st8["ps"] = proj_ps.tile([P, CHW], F32, tag="proj", name=f"ps_proj_{id(st8)}_{c}")
                        nc.tensor.matmul(
                            st8["ps"][:], w_t["Wq"][:, c, :], xf(c),
                            start=(c == 0), stop=(c == CT - 1),
                        )
                    return run

                def glue():
                    qt = q_pool.tile([P, CHW], F32R, tag="qch", name=f"q_{j}")
                    nc.vector.tensor_scalar(
                        qt[:], st8["ps"][:], bq_sb[:], None, ADD,
                    )
                    q_ch[j] = qt

                return [mk_mm(c) for c in range(CT)] + [glue]

            def proj_k_thunks(j, xf):
                st8 = {}

                def mk_mm(c):
                    def run():
                        if c == 0:
                            st8["ps"] = proj_ps.tile([P, CHW], F32, tag="proj", name=f"ps_proj_{id(st8)}_{c}")
                        nc.tensor.matmul(
                            st8["ps"][:], w_t["Wk"][:, c, :], xf(c),
                            start=(c == 0), stop=(c == CT - 1),
                        )
                    return run

                def glue():
                    kt = kt_pool.tile([P, CHW], F32R, tag=f"kt{j}", name=f"kt_{j}")
                    nc.scalar.copy(kt[:], st8["ps"][:])
                    kt_ch[j] = kt

                return [mk_mm(c) for c in range(CT)] + [glue]

            def proj_v_thunks(j, xf):
                st8 = {}

                def mk_mm(ss, c):
                    def run():
                        if ss == 0 and c == 0:
                            st8["ps"] = proj_ps.tile([P, CHW], F32, tag="proj", name=f"vps_{j}")
                        nc.tensor.matmul(
                            st8["ps"][:, P * ss : P * (ss + 1)],
                            xf(c, ss),
                            w_t["Wv"][:, c, :],
                            start=(c == 0), stop=(c == CT - 1),
                            skip_group_check=True,
                        )
                    return run

                def vncopy():
                    vn = vn_pool.tile([P, CHW], BF16, tag=f"vn{j}", name=f"vn_{j}")
                    nc.scalar.copy(vn[:], st8["ps"][:])
                    v_nat[j] = vn

                return (
                    [mk_mm(ss, c) for ss in range(SPC) for c in range(CT)]
                    + [vncopy]
                )

            chunk_st = {}

            def attn_begin(j):
                st = {
                    "acc": acc_pool.tile([P, CHW], BF16, tag="acc", name=f"acc_{j}"),
                    "ps_o": o_ps.tile([P, CHW], F32, tag="o", name=f"ps_o_{j}"),
                    "ems": [],
                    "n": SPC * (j + 1),
                }
                chunk_st[j] = st
                return st

            def groups_of(j):
                nd = SPC * j
                gs = [(2 * k, 2 * k + 1) for k in range(nd // 2)]
                gs += [(i,) for i in range(nd, SPC * (j + 1))]
                return gs

            def _acc_in(st, em_ap, i, o):
                acc = st["acc"]
                if i == 0:
                    nc.vector.tensor_copy(acc[:], em_ap)
                else:
                    nc.vector.tensor_tensor(acc[:, o:], acc[:, o:], em_ap, ADD)

            def emit_group(j, g):
                st = chunk_st[j]
                if len(g) == 2:
                    # two off-diagonal s-tiles share a 2-bank PSUM tile so one
                    # wide exp covers both (halves ACT's per-op overhead)
                    i0, i1 = g
                    ps2 = sc_ps.tile([P, 2 * CHW], F32, tag="sc", name=f"sc_{j}_{i0}")
                    for k, i in enumerate(g):
                        jj, ss = i // SPC, i % SPC
                        nc.tensor.matmul(
                            ps2[:, CHW * k : CHW * (k + 1)],
                            kt_ch[jj][:, P * ss : P * (ss + 1)],
                            q_ch[j][:],
                            start=True, stop=True, skip_group_check=True,
                        )
                    em2 = e2_pool.tile([P, 2 * CHW], BF16, tag="e2", name=f"em_{j}_{i0}")
                    nc.scalar.activation(em2[:], ps2[:], Exp)
                    _acc_in(st, em2[:, 0:CHW], i0, 0)
                    _acc_in(st, em2[:, CHW : 2 * CHW], i1, 0)
                    st["ems"].append((em2, 0, 0))
                    st["ems"].append((em2, CHW, 0))
                    return
                i = g[0]
                diag = i >= SPC * j
                stt = i - SPC * j
                o = min(P * stt, CHW - MW) if diag else 0
                ps2 = sc_ps.tile([P, 2 * CHW], F32, tag="sc", name=f"sc_{j}_{i}")
                jj, ss = i // SPC, i % SPC
                nc.tensor.matmul(
                    ps2[:, o:CHW],
                    kt_ch[jj][:, P * ss : P * (ss + 1)],
                    q_ch[j][:, o:],
                    start=True, stop=True, skip_group_check=True,
                )
                em = e_pool.tile([P, CHW], BF16, tag="e", name=f"em_{j}_{i}")
                nc.scalar.activation(em[:, o:], ps2[:, o:CHW], Exp)
                if diag:
                    # clamped tile (stt==SPC-1): zero the over-computed
                    # columns too -> 256-wide [zeros|tri] mask at o;
                    # otherwise just the 128-wide triangle at its block
                    mo, mw = (o, MW) if stt == SPC - 1 else (P * stt, P)
                    nc.vector.tensor_tensor(
                        em[:, mo : mo + mw], em[:, mo : mo + mw],
                        mi[:, MW * stt : MW * stt + mw], MULT,
                    )
                _acc_in(st, em[:, o:], i, o)
                st["ems"].append((em, 0, o))

            def emit_out(j, i):
                st = chunk_st[j]
                em, base, o = st["ems"][i]
                jj, ss = i // SPC, i % SPC
                nc.tensor.matmul(
                    st["ps_o"][:, o:],
                    v_nat[jj][:, P * ss : P * (ss + 1)],
                    em[:, base + o : base + CHW],
                    start=(i == 0), stop=(i == st["n"] - 1),
                    skip_group_check=True,
                )

            def make_epilogue(j, halves=1):
                st = chunk_st[j]

                def epilogue():
                    ps_d = proj_ps.tile([P, CHW], F32, tag="proj", name=f"ps_d_{j}")
                    recip = out_pool.tile([P, CHW], F32, tag="recip", name=f"recip_{j}")
                    o1 = out_pool.tile([P, CHW], F32, tag="o1", name=f"o1_{j}")
                    hw_ = CHW // halves
                    for h in range(halves):
                        hs = slice(hw_ * h, hw_ * (h + 1))
                        nc.tensor.matmul(
                            ps_d[:, hs], ones_r[:], st["acc"][:, hs],
                            start=True, stop=True, skip_group_check=True,
                        )
                        nc.vector.reciprocal_approx_fast(
                            out=recip[:, hs], in_=ps_d[:, hs],
                        )
                        nc.vector.tensor_tensor(
                            o1[:, hs], st["ps_o"][:, hs], recip[:, hs], MULT,
                        )
                        nc.sync.dma_start(
                            out_ext[:, CHW * j + hw_ * h : CHW * j + hw_ * (h + 1)],
                            o1[:, hs],
                        )

                return epilogue

            def attn(j, pending_epilogue=None, pre=0):
                st = chunk_st[j] if pre else attn_begin(j)
                n = st["n"]
                lag = n if j == 0 else 2
                gs = groups_of(j)
                done = 0
                while done < pre:
                    done += len(gs.pop(0))
                anchors = max(1, len(gs) + n)
                q = -(-len(filler) // anchors)  # ceil: drain evenly across chunk
                oi = 0
                for g in gs:
                    emit_group(j, g)
                    done += len(g)
                    pop_fillers(q)
                    if pending_epilogue is not None:
                        pending_epilogue()
                        pending_epilogue = None
                    while oi <= done - 1 - lag:
                        emit_out(j, oi)
                        oi += 1
                        pop_fillers(q)
                if pending_epilogue is not None:
                    pending_epilogue()
                while oi < n:
                    pop_fillers(q)
                    emit_out(j, oi)
                    oi += 1
                drain_fillers()
                return make_epilogue(j, halves=4 if j == NCH - 1 else 1)

            # ---- chunk 0: q/k projections emitted directly ----
            def xf0(c, ss=None):
                if ss is None:
                    return xt0[:, c, :]
                return xt0[:, c, P * ss : P * (ss + 1)]

            for t in proj_k_thunks(0, xf0):
                t()
            for t in proj_q_thunks(0, xf0):
                t()
            # chunk 0's own v-projection rides the filler queue (drained
            # before the out-matmuls need v_nat[0])
            filler.extend(proj_v_thunks(0, xf0))

            def xf1(c, ss=None):
                if ss is None:
                    return xt1[:, c, :]
                return xt1[:, c, P * ss : P * (ss + 1)]

            for th in (proj_k_thunks(1, xf1), proj_q_thunks(1, xf1),
                       proj_v_thunks(1, xf1)):
                filler.extend(th)
            pend = attn(0)
            PRE = {2: 4, 3: 6}
            for j in range(1, NCH):
                if j < NCH - 1:
                    xt = x23_tiles[j + 1]

                    def xfn(c, ss=None, _xt=xt):
                        if ss is None:
                            return _xt[:, c, :]
                        return _xt[:, c, P * ss : P * (ss + 1)]

                    for th in (proj_k_thunks(j + 1, xfn), proj_q_thunks(j + 1, xfn),
                               proj_v_thunks(j + 1, xfn)):
                        filler.extend(th)
                pend = attn(j, pend, pre=PRE.get(j, 0))
                # work ahead: emit the next chunk's off-diagonal scores now so
                # ACT isn't the serial pacer when that chunk's outs run
                if j + 1 in PRE:
                    attn_begin(j + 1)
                    cnt = 0
                    for g in groups_of(j + 1):
                        if cnt >= PRE[j + 1]:
                            break
                        emit_group(j + 1, g)
                        cnt += len(g)
            pend()

    nc.compile()
    _BUILT = nc
    return nc


def _host_inputs(x, Wq, bq, Wk, bk, Wv, bv):
    import ml_dtypes

    bf16 = ml_dtypes.bfloat16
    # masks[p, st, u] over the uniform 256-wide region starting at
    # o_c = min(128*st, 256) of the diagonal s-tile st:
    #   st<3: keep if u>=128 or p<=u ; st=3: keep if u>=128 and p<=u-128
    ps = np.arange(P)[:, None, None]
    stv = np.arange(SPC)[None, :, None]
    u = np.arange(MW)[None, None, :]
    m_lo = (u >= P) | (ps <= u)          # st < 3
    m_hi = (u >= P) & (ps <= u - P)      # st = 3
    masks = np.where(stv < SPC - 1, m_lo, m_hi).astype(bf16)
    mi = masks.reshape(P, SPC * MW)

    shared = {
        "Wq": np.ascontiguousarray(Wq, dtype=np.float16),
        "Wk": np.ascontiguousarray(Wk, dtype=np.float16),
        "Wv": np.ascontiguousarray(Wv, dtype=np.float16),
        "bq": np.ascontiguousarray(bq, dtype=np.float32).reshape(H, 1),
        "masks": mi,
    }
    in_maps = []
    for b in range(B):
        m = dict(shared)
        m["xT"] = np.ascontiguousarray(np.asarray(x[b]).T.astype(np.float16))
        in_maps.append(m)
    return in_maps


def kernel(x, Wq, bq, Wk, bk, Wv, bv):
    global LAST_EXEC_TIME_NS
    from concourse.bass_utils import run_bass_kernel_spmd

    nc = _build()
    in_maps = _host_inputs(x, Wq, bq, Wk, bk, Wv, bv)
    trace = os.environ.get("BASS_ATTN_TRACE", "0") == "1"
    res = run_bass_kernel_spmd(nc, in_maps, core_ids=list(range(N_CORES)), trace=trace)
    LAST_EXEC_TIME_NS = res.exec_time_ns
    out = np.stack([res.results[b]["out"].T for b in range(B)], axis=0)
    out += np.asarray(bv, dtype=np.float32)[None, None, :]
    return np.ascontiguousarray(out, dtype=np.float32)


# revision 31
# speedup vs baseline: 1.0322x; 1.0322x over previous
"""Single-head causal attention (B=8, T=2048, C=1024, H=128) on 8 TRN2 NeuronCores.

Sharding: data-parallel over batch — core b computes batch element b entirely
(no collectives). Host pre-transposes x[b] to xT=[C,T] in float16 (10 mantissa
bits keep logit error ~7e-3; bf16 would not); the device returns out^T=[H,T]
which the host transposes back and adds bv to.

Per-core dataflow (T split into 4 chunks of 512 columns):
  qT/kT chunk   = sum_c W[c-tile].T @ xT[c-tile]    (fp16 matmuls, f32 PSUM)
  bq added on DVE; bk dropped (softmax is invariant to per-query shifts);
  bv applied on the host after the gather.
  v_nat[s-tile] = sum_c xT[c,s-slice].T @ Wv[c]     (fp16, natural [s,h] layout
                  directly — fp16 runs 1 cyc/row at any free size, so no
                  PE transpose pass is needed)
  scoresT[s,t]  = kT-tile.T-as-lhsT @ qT-chunk      (float32r, causal-trimmed,
                  diagonal tiles clamped to >=256 free so fp32r stays 1 cyc/row)
  expT          = Exp(scoresT) on ACT -> bf16, diagonal blocks masked on DVE
  denom         = DVE-accumulated bf16 expT tiles + one ones-matmul per chunk
  out^T        += v_nat[i] @ expT                   (bf16 matmuls)
  out^T[:,chunk]= out^T * 1/denom  -> DMA out

Scheduling notes (the perf comes from these):
  - one dma_start per logical tensor (each costs ~0.7us of serial Sync time);
    all four x-chunks are enqueued in the prologue (SBUF-resident).
  - warmup matmuls bridge the DMA prologue so the PE clock (DVFS) is ramped
    when real work starts; any PE idle gap downclocks and costs double.
  - chunk j+1's projections ride a filler queue interleaved into chunk j's
    exp-latency-bound attention ops, drained adaptively; each chunk's
    denominator/normalize epilogue is deferred into the next chunk's stream;
    the next chunk's off-diagonal scores are pre-emitted (PRE) so ACT is never
    the serial pacer at a chunk boundary.
"""

import os
from collections import deque

import numpy as np

T, C, H = 2048, 1024, 128
B = 8
P = 128
CT = C // P          # 8 contraction tiles
NCH = 4              # t-chunks
CHW = T // NCH       # 512 chunk width
SPC = CHW // P       # 4 s-tiles per chunk
MW = 256             # uniform mask width
N_CORES = 8
N_WARM = 26

LAST_EXEC_TIME_NS = None

_BUILT = None


def _build():
    global _BUILT
    if _BUILT is not None:
        return _BUILT

    import concourse.bass as bass  # noqa: F401
    import concourse.mybir as mybir
    from concourse import bacc
    from concourse.tile import TileContext

    F32 = mybir.dt.float32
    F32R = mybir.dt.float32r
    F16 = mybir.dt.float16
    BF16 = mybir.dt.bfloat16
    Exp = mybir.ActivationFunctionType.Exp
    ADD = mybir.AluOpType.add
    MULT = mybir.AluOpType.mult

    nc = bacc.Bacc()

    xT_ext = nc.declare_dram_parameter("xT", [C, T], F16, isOutput=False)
    w_ext = {
        n: nc.declare_dram_parameter(n, [C, H], F16, isOutput=False)
        for n in ("Wq", "Wk", "Wv")
    }
    bq_ext = nc.declare_dram_parameter("bq", [H, 1], F32, isOutput=False)
    masks_ext = nc.declare_dram_parameter("masks", [P, SPC * MW], BF16, isOutput=False)
    out_ext = nc.declare_dram_parameter("out", [H, T], F32, isOutput=True)

    xT_r = xT_ext.rearrange("(ct p) t -> p ct t", p=P)
    w_r = {n: w_ext[n].rearrange("(ct p) h -> p ct h", p=P) for n in w_ext}

    with TileContext(nc) as tc:
        with (
            tc.tile_pool(name="const", bufs=1) as const,
            tc.tile_pool(name="xch", bufs=3) as x_pool,
            tc.tile_pool(name="qch", bufs=2) as q_pool,
            tc.tile_pool(name="ktc", bufs=4) as kt_pool,
            tc.tile_pool(name="vn", bufs=4) as vn_pool,
            tc.tile_pool(name="ex", bufs=20) as e_pool,
            tc.tile_pool(name="accp", bufs=2) as acc_pool,
            tc.tile_pool(name="outp", bufs=2) as out_pool,
            tc.tile_pool(name="ps_proj", bufs=2, space="PSUM") as proj_ps,
            tc.tile_pool(name="ps_sc", bufs=3, space="PSUM") as sc_ps,
            tc.tile_pool(name="ps_o", bufs=2, space="PSUM") as o_ps,
            tc.tile_pool(name="ps_tr", bufs=1, space="PSUM") as tr_ps,
        ):
            # ---- constants / prologue DMAs (priority order) ----
            # batched transfers: one dma_start per logical tensor, because
            # every dma_start costs ~0.7us of serial Sync-engine issue time
            w_t = {}
            w_t["Wq"] = const.tile([P, CT, H], F16, tag="w_Wq", name="w_Wq")
            nc.sync.dma_start(w_t["Wq"][:], w_r["Wq"][:])
            xt0 = const.tile([P, CT, CHW], F16, tag="x0", name="x_0")
            nc.sync.dma_start(xt0[:], xT_r[:, :, 0:CHW])
            bq_sb = const.tile([H, 1], F32, tag="bq", name="bq_sb")
            nc.sync.dma_start(bq_sb[:], bq_ext[:])
            w_t["Wk"] = const.tile([P, CT, H], F16, tag="w_Wk", name="w_Wk")
            nc.sync.dma_start(w_t["Wk"][:], w_r["Wk"][:])
            mi = const.tile([P, SPC * MW], BF16, tag="mi", name="mi")
            nc.sync.dma_start(mi[:], masks_ext[:])
            xt1 = x_pool.tile([P, CT, CHW], F16, tag="x", name="x_1")
            nc.sync.dma_start(xt1[:], xT_r[:, :, CHW : 2 * CHW])
            w_t["Wv"] = const.tile([P, CT, H], F16, tag="w_Wv", name="w_Wv")
            nc.sync.dma_start(w_t["Wv"][:], w_r["Wv"][:])
            ones_r = const.tile([P, P], BF16, tag="ones_r", name="ones_r")
            nc.vector.memset(ones_r[:], 1.0)
            x23_tiles = {}
            for jj in (2, 3):
                xt = x_pool.tile([P, CT, CHW], F16, tag="x", name=f"x_{jj}")
                nc.sync.dma_start(xt[:], xT_r[:, :, CHW * jj : CHW * (jj + 1)])
                x23_tiles[jj] = xt
            warm_src = const.tile([P, CHW], BF16, tag="warm_src", name="warm_src")
            nc.vector.memset(warm_src[:], 0.0)

            # PE warmup: dummy matmuls spanning the DMA prologue so HAM is at
            # full clock when the first real matmul issues.
            ps_warm = proj_ps.tile([P, CHW], F32, tag="proj", name="ps_warm")
            for _w in range(N_WARM):
                nc.tensor.matmul(
                    ps_warm[:], warm_src[:, 0:P], warm_src[:], start=True, stop=True,
                )

            kt_ch = [None] * NCH   # [h=128, s=512] f32r per chunk
            v_nat = [None] * NCH   # [s=128, st*128+h] bf16 per chunk
            q_ch = [None] * NCH    # [h=128, t=512] f32r per chunk

            filler = deque()

            def pop_fillers(k):
                for _ in range(min(k, len(filler))):
                    filler.popleft()()

            def drain_fillers():
                while filler:
                    filler.popleft()()

            def proj_q_thunks(j, xf):
                st8 = {}

                def mk_mm(c):
                    def run():
                        if c == 0:
                            st8["ps"] = proj_ps.tile([P, CHW], F32, tag="proj", name=f"ps_proj_{id(st8)}_{c}")
                        nc.tensor.matmul(
                            st8["ps"][:], w_t["Wq"][:, c, :], xf(c),
                            start=(c == 0), stop=(c == CT - 1),
                        )
                    return run

                def glue():
                    qt = q_pool.tile([P, CHW], F32R, tag="qch", name=f"q_{j}")
                    nc.vector.tensor_scalar(
                        qt[:], st8["ps"][:], bq_sb[:], None, ADD,
                    )
                    q_ch[j] = qt

                return [mk_mm(c) for c in range(CT)] + [glue]

            def proj_k_thunks(j, xf):
                st8 = {}

                def mk_mm(c):
                    def run():
                        if c == 0:
                            st8["ps"] = proj_ps.tile([P, CHW], F32, tag="proj", name=f"ps_proj_{id(st8)}_{c}")
                        nc.tensor.matmul(
                            st8["ps"][:], w_t["Wk"][:, c, :], xf(c),
                            start=(c == 0), stop=(c == CT - 1),
                        )
                    return run

                def glue():
                    kt = kt_pool.tile([P, CHW], F32R, tag=f"kt{j}", name=f"kt_{j}")
                    nc.scalar.copy(kt[:], st8["ps"][:])
                    kt_ch[j] = kt

                return [mk_mm(c) for c in range(CT)] + [glue]

            def proj_v_thunks(j, xf):
                st8 = {}

                def mk_mm(ss, c):
                    def run():
                        if ss == 0 and c == 0:
                            st8["ps"] = tr_ps.tile([P, CHW], F32, tag="tr", name=f"vps_{j}")
                        nc.tensor.matmul(
                            st8["ps"][:, P * ss : P * (ss + 1)],
                            xf(c, ss),
                            w_t["Wv"][:, c, :],
                            start=(c == 0), stop=(c == CT - 1),
                            skip_group_check=True,
                        )
                    return run

                def vncopy():
                    vn = vn_pool.tile([P, CHW], BF16, tag=f"vn{j}", name=f"vn_{j}")
                    nc.scalar.copy(vn[:], st8["ps"][:])
                    v_nat[j] = vn

                return (
                    [mk_mm(ss, c) for ss in range(SPC) for c in range(CT)]
                    + [vncopy]
                )

            chunk_st = {}

            def attn_begin(j):
                st = {
                    "acc": acc_pool.tile([P, CHW], BF16, tag="acc", name=f"acc_{j}"),
                    "ps_o": o_ps.tile([P, CHW], F32, tag="o", name=f"ps_o_{j}"),
                    "ems": [],
                    "n": SPC * (j + 1),
                }
                chunk_st[j] = st
                return st

            def emit_score(j, i):
                st = chunk_st[j]
                diag = i >= SPC * j
                stt = i - SPC * j
                o = min(P * stt, CHW - MW) if diag else 0
                ps_sc = sc_ps.tile([P, CHW], F32, tag="sc", name=f"ps_sc_{j}_{i}")
                jj, ss = i // SPC, i % SPC
                nc.tensor.matmul(
                    ps_sc[:, o:],
                    kt_ch[jj][:, P * ss : P * (ss + 1)],
                    q_ch[j][:, o:],
                    start=True, stop=True,
                )
                em = e_pool.tile([P, CHW], BF16, tag="e", name=f"em_{j}_{i}")
                nc.scalar.activation(em[:, o:], ps_sc[:, o:], Exp)
                if diag:
                    # clamped tile (stt==SPC-1): zero the over-computed
                    # columns too -> 256-wide [zeros|tri] mask at o;
                    # otherwise just the 128-wide triangle at its block
                    mo, mw = (o, MW) if stt == SPC - 1 else (P * stt, P)
                    nc.vector.tensor_tensor(
                        em[:, mo : mo + mw], em[:, mo : mo + mw],
                        mi[:, MW * stt : MW * stt + mw], MULT,
                    )
                acc = st["acc"]
                if i == 0:
                    nc.vector.tensor_copy(acc[:], em[:])
                else:
                    nc.vector.tensor_tensor(
                        acc[:, o:], acc[:, o:], em[:, o:], ADD,
                    )
                st["ems"].append((em, o))

            def emit_out(j, i):
                st = chunk_st[j]
                em, o = st["ems"][i]
                jj, ss = i // SPC, i % SPC
                nc.tensor.matmul(
                    st["ps_o"][:, o:],
                    v_nat[jj][:, P * ss : P * (ss + 1)],
                    em[:, o:],
                    start=(i == 0), stop=(i == st["n"] - 1),
                    skip_group_check=True,
                )

            def make_epilogue(j, halves=1):
                st = chunk_st[j]

                def epilogue():
                    ps_d = proj_ps.tile([P, CHW], F32, tag="proj", name=f"ps_d_{j}")
                    recip = out_pool.tile([P, CHW], F32, tag="recip", name=f"recip_{j}")
                    o1 = out_pool.tile([P, CHW], F32, tag="o1", name=f"o1_{j}")
                    hw_ = CHW // halves
                    for h in range(halves):
                        hs = slice(hw_ * h, hw_ * (h + 1))
                        nc.tensor.matmul(
                            ps_d[:, hs], ones_r[:], st["acc"][:, hs],
                            start=True, stop=True, skip_group_check=True,
                        )
                        nc.vector.reciprocal_approx_fast(
                            out=recip[:, hs], in_=ps_d[:, hs],
                        )
                        nc.vector.tensor_tensor(
                            o1[:, hs], st["ps_o"][:, hs], recip[:, hs], MULT,
                        )
                        nc.sync.dma_start(
                            out_ext[:, CHW * j + hw_ * h : CHW * j + hw_ * (h + 1)],
                            o1[:, hs],
                        )

                return epilogue

            def attn(j, pending_epilogue=None, pre=0):
                st = chunk_st[j] if pre else attn_begin(j)
                n = st["n"]
                lag = n if j == 0 else 2
                anchors = max(1, 2 * n - pre)
                q = -(-len(filler) // anchors)  # ceil: drain evenly across chunk
                oi = 0
                for si in range(pre, n):
                    emit_score(j, si)
                    pop_fillers(q)
                    if si == max(pre, 1) and pending_epilogue is not None:
                        pending_epilogue()
                        pending_epilogue = None
                    while oi <= si - lag:
                        emit_out(j, oi)
                        oi += 1
                        pop_fillers(q)
                if pending_epilogue is not None:
                    pending_epilogue()
                while oi < n:
                    pop_fillers(q)
                    emit_out(j, oi)
                    oi += 1
                drain_fillers()
                return make_epilogue(j, halves=2 if j == NCH - 1 else 1)

            # ---- chunk 0: q/k projections emitted directly ----
            def xf0(c, ss=None):
                if ss is None:
                    return xt0[:, c, :]
                return xt0[:, c, P * ss : P * (ss + 1)]

            for t in proj_k_thunks(0, xf0):
                t()
            for t in proj_q_thunks(0, xf0):
                t()
            # chunk 0's own v-projection rides the filler queue (drained
            # before the out-matmuls need v_nat[0])
            filler.extend(proj_v_thunks(0, xf0))

            def xf1(c, ss=None):
                if ss is None:
                    return xt1[:, c, :]
                return xt1[:, c, P * ss : P * (ss + 1)]

            for th in (proj_k_thunks(1, xf1), proj_q_thunks(1, xf1),
                       proj_v_thunks(1, xf1)):
                filler.extend(th)
            pend = attn(0)
            PRE = {2: 4, 3: 6}
            for j in range(1, NCH):
                if j < NCH - 1:
                    xt = x23_tiles[j + 1]

                    def xfn(c, ss=None, _xt=xt):
                        if ss is None:
                            return _xt[:, c, :]
                        return _xt[:, c, P * ss : P * (ss + 1)]

                    for th in (proj_k_thunks(j + 1, xfn), proj_q_thunks(j + 1, xfn),
                               proj_v_thunks(j + 1, xfn)):
                        filler.extend(th)
                pend = attn(j, pend, pre=PRE.get(j, 0))
                # work ahead: emit the next chunk's off-diagonal scores now so
                # ACT isn't the serial pacer when that chunk's outs run
                if j + 1 in PRE:
                    attn_begin(j + 1)
                    for i in range(PRE[j + 1]):
                        emit_score(j + 1, i)
            pend()

    nc.compile()
    _BUILT = nc
    return nc


def _host_inputs(x, Wq, bq, Wk, bk, Wv, bv):
    import ml_dtypes

    bf16 = ml_dtypes.bfloat16
    # masks[p, st, u] over the uniform 256-wide region starting at
    # o_c = min(128*st, 256) of the diagonal s-tile st:
    #   st<3: keep if u>=128 or p<=u ; st=3: keep if u>=128 and p<=u-128
    ps = np.arange(P)[:, None, None]
    stv = np.arange(SPC)[None, :, None]
    u = np.arange(MW)[None, None, :]
    m_lo = (u >= P) | (ps <= u)          # st < 3
    m_hi = (u >= P) & (ps <= u - P)      # st = 3
    masks = np.where(stv < SPC - 1, m_lo, m_hi).astype(bf16)
    mi = masks.reshape(P, SPC * MW)

    shared = {
        "Wq": np.ascontiguousarray(Wq, dtype=np.float16),
        "Wk": np.ascontiguousarray(Wk, dtype=np.float16),
        "Wv": np.ascontiguousarray(Wv, dtype=np.float16),
        "bq": np.ascontiguousarray(bq, dtype=np.float32).reshape(H, 1),
        "masks": mi,
    }
    in_maps = []
    for b in range(B):
        m = dict(shared)
        m["xT"] = np.ascontiguousarray(np.asarray(x[b]).T.astype(np.float16))
        in_maps.append(m)
    return in_maps


def kernel(x, Wq, bq, Wk, bk, Wv, bv):
    global LAST_EXEC_TIME_NS
    from concourse.bass_utils import run_bass_kernel_spmd

    nc = _build()
    in_maps = _host_inputs(x, Wq, bq, Wk, bk, Wv, bv)
    trace = os.environ.get("BASS_ATTN_TRACE", "0") == "1"
    res = run_bass_kernel_spmd(nc, in_maps, core_ids=list(range(N_CORES)), trace=trace)
    LAST_EXEC_TIME_NS = res.exec_time_ns
    out = np.stack([res.results[b]["out"].T for b in range(B)], axis=0)
    out += np.asarray(bv, dtype=np.float32)[None, None, :]
    return np.ascontiguousarray(out, dtype=np.float32)
